# revision 1
# baseline (speedup 1.0000x reference)
"""MoE (top-2 of 8 experts, SwiGLU) Trainium2 kernel.

Strategy (expert-parallel, host-orchestrated dispatch):
  - Host computes routing (top-2 expert ids per token) from the gate logits
    and gathers each expert's tokens into a fixed-capacity buffer.
  - 8 NeuronCores run SPMD: core e holds expert e's weights, computes
      h = silu(x @ w1) * (x @ w3);  outT = (h @ w2)^T
    for its gathered tokens, plus a 1/8 slice of the gate logits
    (gate is data-parallel across cores).
  - Host combines: softmax over device-computed logits -> top-2 renormalized
    weights -> weighted scatter-add of per-expert outputs.

Layouts: activations are stored transposed (feature dim on partitions,
tokens on the free dim) so both matmul stages keep weights stationary:
  phase A: Ht[h, t]  = sum_d w1[d, h] * xT[d, t]   (lhsT = w1 tile)
  phase B: outT[d,t] = sum_h w2[h, d] * Ht[h, t]   (lhsT = w2 tile)
The gate always runs in float32r; the MLP dtype is MOE_DTYPE (f32r | bf16).
"""

import os
from contextlib import ExitStack

import ml_dtypes
import numpy as np

import concourse.tile as tile
from concourse import bacc, mybir
import concourse.bass_utils as _bu
from concourse.bass_utils import run_bass_kernel_spmd

# If a caller enables BASS_TRACE, the trace path uploads NTFF artifacts to a
# shared bucket; containers without bucket access would crash the whole run.
# Fall back to the local tmpdir so tracing still completes.
_orig_upload = _bu.upload_artifacts


def _safe_upload(tmpdir):
    try:
        return _orig_upload(tmpdir)
    except Exception:
        return tmpdir


_bu.upload_artifacts = _safe_upload

P = 128
D = 1024
H = 4096
E = 8
T = 4096
TG = T // E  # gate tokens per core (data-parallel gate)
HB = 256     # H block size (weights streamed block-by-block)
F32 = mybir.dt.float32
F32R = mybir.dt.float32r
BF16 = mybir.dt.bfloat16
SIGMOID = mybir.ActivationFunctionType.Sigmoid
SILU = mybir.ActivationFunctionType.Silu
# CoreSim does not implement Silu; set MOE_SIM_SAFE=1 to emit sigmoid*x.
_SIM_SAFE = os.environ.get("MOE_SIM_SAFE") == "1"
# MLP matmul dtype: "f32r" (default, ~2.8e-4 rel err) or "bf16" (faster)
_DTYPE = os.environ.get("MOE_DTYPE", "f32r")


def _mlp_dt():
    return BF16 if _DTYPE == "bf16" else F32R


def _np_mlp_dt():
    return ml_dtypes.bfloat16 if _DTYPE == "bf16" else np.float32


def _chunks_of(c):
    """Split capacity C into near-equal matmul free-dim chunks.

    Each chunk is a multiple of 128 in [256, 512]; near-equal sizes keep
    every matmul's streaming time at or above the LDWEIGHTS cost.
    """
    if c <= 0 or c % 128 != 0:
        raise ValueError(f"bad capacity {c}")
    n = -(-c // 512)
    t = c // 128
    base, extra = divmod(t, n)
    out = [128 * (base + (1 if i < extra else 0)) for i in range(n)]
    if out[-1] < 256:  # only possible for c < 256
        raise ValueError(f"bad capacity {c}")
    return out


def _ld(ap, dt):
    """DRAM-side AP for a weight/activation load at the MLP dtype."""
    return ap.bitcast(dt) if dt == F32R else ap


def _moe_body(ctx, tc, aps, C, chunks):
    nc = tc.nc
    MDT = _mlp_dt()
    DT = D // P        # 8 d-tiles
    HT = HB // P       # h-tiles per block
    NHB = H // HB      # number of H blocks
    xg, wg, xc, w1, w3, w2, logits_o, outT_o = (
        aps["xg"], aps["wg"], aps["xc"], aps["w1"], aps["w3"], aps["w2"],
        aps["logits"], aps["outT"])

    const = ctx.enter_context(tc.tile_pool(name="const", bufs=1))
    xc_pool = ctx.enter_context(tc.tile_pool(name="xc", bufs=1))
    acc_pool = ctx.enter_context(tc.tile_pool(name="acc", bufs=1))
    wpool = ctx.enter_context(tc.tile_pool(name="w", bufs=2))
    htpool = ctx.enter_context(tc.tile_pool(name="ht", bufs=2))
    stage = ctx.enter_context(tc.tile_pool(name="stage", bufs=4))
    psA = ctx.enter_context(tc.tile_pool(name="psA", bufs=4, space="PSUM"))
    psB = ctx.enter_context(tc.tile_pool(name="psB", bufs=3, space="PSUM"))

    engs = [nc.sync, nc.gpsimd, nc.scalar]

    offs = []
    o = 0
    for ck in chunks:
        offs.append((o, ck))
        o += ck

    # ---- persistent activations ----
    # Chunk-progressive loads across queues: the first phase-A unit only
    # needs chunk 0 of every d-tile, so those 8 slices land first.
    xc_t = [xc_pool.tile([P, C], MDT, tag=f"xc{d}", name=f"xc{d}")
            for d in range(DT)]
    for (c0, ck) in offs:
        for d in range(DT):
            engs[d % 3].dma_start(
                xc_t[d][:, c0:c0 + ck],
                _ld(xc[d * P:(d + 1) * P, c0:c0 + ck], MDT))
    acc_t = [acc_pool.tile([P, C], F32, tag=f"acc{d}", name=f"acc{d}")
             for d in range(DT)]

    # gate inputs prefetched on the scalar queue; consumed at the end
    wg_t = [const.tile([P, E], F32R, tag=f"wg{d}", name=f"wg{d}")
            for d in range(DT)]
    xg_t = [const.tile([P, TG], F32R, tag=f"xg{d}", name=f"xg{d}")
            for d in range(DT)]
    for d in range(DT):
        nc.scalar.dma_start(wg_t[d][:], wg[d * P:(d + 1) * P, :].bitcast(F32R))
        nc.scalar.dma_start(xg_t[d][:], xg[d * P:(d + 1) * P, :].bitcast(F32R))

    for hb in range(NHB):
        h0 = hb * HB
        # stream this H block's weights
        w1_t = [wpool.tile([P, HB], MDT, tag=f"w1_{d}", name=f"w1t{d}")
                for d in range(DT)]
        w3_t = [wpool.tile([P, HB], MDT, tag=f"w3_{d}", name=f"w3t{d}")
                for d in range(DT)]
        for d in range(DT):
            nc.sync.dma_start(w1_t[d][:],
                              _ld(w1[d * P:(d + 1) * P, h0:h0 + HB], MDT))
            nc.gpsimd.dma_start(w3_t[d][:],
                                _ld(w3[d * P:(d + 1) * P, h0:h0 + HB], MDT))
        w2_t = [wpool.tile([P, D], MDT, tag=f"w2_{k}", name=f"w2t{k}")
                for k in range(HT)]
        for k in range(HT):
            nc.scalar.dma_start(w2_t[k][:],
                                _ld(w2[h0 + k * P:h0 + (k + 1) * P, :], MDT))

        # phase A: Ht[h, t] = silu(w1.T @ x) * (w3.T @ x) for this block
        ht_t = [htpool.tile([P, C], MDT, tag=f"ht{k}", name=f"htt{k}")
                for k in range(HT)]
        for (c0, ck) in offs:
            for k in range(HT):
                hsl = slice(k * P, (k + 1) * P)
                p1 = psA.tile([P, ck], F32, tag="p1", name="p1", bufs=3)
                p3 = psA.tile([P, ck], F32, tag="p3", name="p3", bufs=2)
                for d in range(DT):
                    nc.tensor.matmul(
                        p1[:], w1_t[d][:, hsl], xc_t[d][:, c0:c0 + ck],
                        start=(d == 0), stop=(d == DT - 1))
                for d in range(DT):
                    nc.tensor.matmul(
                        p3[:], w3_t[d][:, hsl], xc_t[d][:, c0:c0 + ck],
                        start=(d == 0), stop=(d == DT - 1))
                sil = stage.tile([P, ck], F32, tag="sil", name="sil")
                if _SIM_SAFE:
                    nc.scalar.activation(sil[:], p1[:], SIGMOID)
                    nc.vector.tensor_mul(sil[:], sil[:], p1[:])
                else:
                    nc.scalar.activation(sil[:], p1[:], SILU)
                nc.vector.tensor_mul(ht_t[k][:, c0:c0 + ck], sil[:], p3[:])

        if hb == 1:
            # gate compute tucked mid-pipeline (inputs prefetched at start;
            # always fp32r for logit precision)
            ps_g = psB.tile([E, TG], F32, tag="pb", name="psg")
            for d in range(DT):
                nc.tensor.matmul(ps_g[:], wg_t[d][:], xg_t[d][:],
                                 start=(d == 0), stop=(d == DT - 1))
            lg_s = const.tile([E, TG], F32, tag="lg", name="lg")
            nc.scalar.copy(lg_s[:], ps_g[:])
            nc.sync.dma_start(logits_o[:, :], lg_s[:])

        # phase B: outT[d, t] += w2.T @ Ht for this block
        for dt in range(DT):
            dsl = slice(dt * P, (dt + 1) * P)
            for (c0, ck) in offs:
                pb = psB.tile([P, ck], F32, tag="pb", name="pb", bufs=3)
                for k in range(HT):
                    nc.tensor.matmul(
                        pb[:], w2_t[k][:, dsl], ht_t[k][:, c0:c0 + ck],
                        start=(k == 0), stop=(k == HT - 1))
                if hb == 0:
                    nc.vector.tensor_copy(acc_t[dt][:, c0:c0 + ck], pb[:])
                else:
                    nc.vector.tensor_add(acc_t[dt][:, c0:c0 + ck],
                                         acc_t[dt][:, c0:c0 + ck], pb[:])

    for d in range(DT):
        nc.sync.dma_start(outT_o[d * P:(d + 1) * P, :], acc_t[d][:])


_NC_CACHE = {}
_LAST_EXEC_NS = None
_LAST_BR = None


def _build_nc(C):
    key = (C, _DTYPE)
    if key in _NC_CACHE:
        return _NC_CACHE[key]
    chunks = _chunks_of(C)
    mdt = F32 if _DTYPE == "f32r" else BF16
    nc = bacc.Bacc("TRN2", target_bir_lowering=False, debug=False,
                   num_devices=E)
    aps = {}
    for name, shape, dt in [("xg", [D, TG], F32), ("wg", [D, E], F32),
                            ("xc", [D, C], mdt), ("w1", [D, H], mdt),
                            ("w3", [D, H], mdt), ("w2", [H, D], mdt)]:
        aps[name] = nc.dram_tensor(name, shape, dt, kind="ExternalInput").ap()
    for name, shape in [("logits", [E, TG]), ("outT", [D, C])]:
        aps[name] = nc.dram_tensor(name, shape, F32, kind="ExternalOutput").ap()
    with tile.TileContext(nc) as tc:
        with ExitStack() as ctx:
            _moe_body(ctx, tc, aps, C, chunks)
    nc.compile()
    _NC_CACHE[key] = nc
    return nc


def kernel(x, wg, w1, w3, w2):
    x = np.asarray(x, np.float32)
    wg = np.asarray(wg, np.float32)
    w1 = np.asarray(w1, np.float32)
    w3 = np.asarray(w3, np.float32)
    w2 = np.asarray(w2, np.float32)
    xt = x.reshape(T, D)
    ndt = _np_mlp_dt()

    # host routing (indices only; combine weights come from device logits)
    lg_h = xt.astype(np.float64) @ wg.astype(np.float64)
    top2 = np.argsort(-lg_h, axis=1)[:, :2]                      # [T, 2]
    idx = [np.nonzero((top2 == e).any(axis=1))[0] for e in range(E)]
    counts = [len(i) for i in idx]
    C = max(512, ((max(counts) + P - 1) // P) * P)

    xT = np.ascontiguousarray(xt.T)                              # [D, T]
    nc = _build_nc(C)
    in_maps = []
    for e in range(E):
        xce = np.zeros((D, C), ndt)
        xce[:, :counts[e]] = xT[:, idx[e]].astype(ndt)
        in_maps.append({
            "xg": np.ascontiguousarray(xT[:, e * TG:(e + 1) * TG]),
            "wg": wg, "xc": xce, "w1": w1[e].astype(ndt, copy=False),
            "w3": w3[e].astype(ndt, copy=False), "w2": w2[e].astype(ndt, copy=False),
        })
    br = run_bass_kernel_spmd(nc, in_maps, list(range(E)))
    global _LAST_EXEC_NS, _LAST_BR
    _LAST_EXEC_NS = br.exec_time_ns
    _LAST_BR = br
    res = br.results

    # combine on host using device-computed gate logits
    lg = np.concatenate([res[e]["logits"].T for e in range(E)], axis=0)
    lg = lg - lg.max(axis=1, keepdims=True)
    p = np.exp(lg)
    p /= p.sum(axis=1, keepdims=True)
    pv = np.take_along_axis(p, top2, axis=1)                     # [T, 2]
    cw = (pv / pv.sum(axis=1, keepdims=True)).astype(np.float32)

    out = np.zeros((T, D), np.float32)
    for e in range(E):
        i = idx[e]
        we = np.where(top2[i, 0] == e, cw[i, 0], cw[i, 1])
        out[i] += we[:, None] * res[e]["outT"][:, :counts[e]].T
    return out.reshape(x.shape)



# revision 2
# speedup vs baseline: 1.0859x; 1.0859x over previous
"""MoE (top-2 of 8 experts, SwiGLU) Trainium2 kernel.

Strategy (expert-parallel, host-orchestrated dispatch):
  - Host computes routing (top-2 expert ids per token) from the gate logits
    and gathers each expert's tokens into a fixed-capacity buffer.
  - 8 NeuronCores run SPMD: core e holds expert e's weights, computes
      h = silu(x @ w1) * (x @ w3);  outT = (h @ w2)^T
    for its gathered tokens, plus a 1/8 slice of the gate logits
    (gate is data-parallel across cores).
  - Host combines: softmax over device-computed logits -> top-2 renormalized
    weights -> weighted scatter-add of per-expert outputs.

Layouts: activations are stored transposed (feature dim on partitions,
tokens on the free dim) so both matmul stages keep weights stationary:
  phase A: Ht[h, t]  = sum_d w1[d, h] * xT[d, t]   (lhsT = w1 tile)
  phase B: outT[d,t] = sum_h w2[h, d] * Ht[h, t]   (lhsT = w2 tile)
The MLP dtype is MOE_DTYPE (bf16 default | f32r); capacity C is the max
expert token count rounded up to 4 (not 128) to minimize padded columns.

Schedule notes (from NTFF trace of the f32r/C=1152 baseline: 28.7us
startup gap waiting on bulk xc+gate DMA ahead of block-0 weights, and a
14.5us tail storing all outputs after the last matmul):
  - DMA order: xc chunk 0 first (4 queues), then block-0/1 weights, then
    the xc remainder, then gate inputs; weight streams run 2 blocks ahead.
  - Final outT stores are interleaved into the last H-block's phase B so
    only the last d-tile's store trails compute.
"""

import os
from contextlib import ExitStack

import ml_dtypes
import numpy as np

import concourse.tile as tile
from concourse import bacc, mybir
import concourse.bass_utils as _bu
from concourse.bass_utils import run_bass_kernel_spmd

# If a caller enables BASS_TRACE, the trace path uploads NTFF artifacts to a
# shared bucket; containers without bucket access would crash the whole run.
# Fall back to the local tmpdir so tracing still completes.
_orig_upload = _bu.upload_artifacts


def _safe_upload(tmpdir):
    try:
        return _orig_upload(tmpdir)
    except Exception:
        return tmpdir


_bu.upload_artifacts = _safe_upload

P = 128
D = 1024
H = 4096
E = 8
T = 4096
TG = T // E  # gate tokens per core (data-parallel gate)
HB = 256     # H block size (weights streamed block-by-block)
GATE_HB = 3  # H block whose slot hides the gate matmul
F32 = mybir.dt.float32
F32R = mybir.dt.float32r
BF16 = mybir.dt.bfloat16
SIGMOID = mybir.ActivationFunctionType.Sigmoid
SILU = mybir.ActivationFunctionType.Silu
# CoreSim does not implement Silu; set MOE_SIM_SAFE=1 to emit sigmoid*x.
_SIM_SAFE = os.environ.get("MOE_SIM_SAFE") == "1"
# MLP matmul dtype: "bf16" (default, ~4.5e-3 rel err) or "f32r" (~2.8e-4)
_DTYPE = os.environ.get("MOE_DTYPE", "bf16")


def _mlp_dt():
    return F32R if _DTYPE == "f32r" else BF16


def _np_mlp_dt():
    return np.float32 if _DTYPE == "f32r" else ml_dtypes.bfloat16


def _chunks_of(c):
    """Split capacity C into near-equal matmul free-dim chunks.

    Each chunk is a multiple of 4 and at most 512 (one f32 PSUM bank);
    near-equal sizes keep LDWEIGHTS hidden under column streaming.
    """
    if c < 512 or c % 4 != 0:
        raise ValueError(f"bad capacity {c}")
    n = -(-c // 512)
    q = c // 4
    base, extra = divmod(q, n)
    return [4 * (base + (1 if i < extra else 0)) for i in range(n)]


def _ld(ap, dt):
    """DRAM-side AP for a weight/activation load at the MLP dtype."""
    return ap.bitcast(dt) if dt == F32R else ap


def _moe_body(ctx, tc, aps, C, chunks):
    nc = tc.nc
    MDT = _mlp_dt()
    DT = D // P        # 8 d-tiles
    HT = HB // P       # h-tiles per block
    NHB = H // HB      # number of H blocks
    xg, wg, xc, w1, w3, w2, logits_o, outT_o = (
        aps["xg"], aps["wg"], aps["xc"], aps["w1"], aps["w3"], aps["w2"],
        aps["logits"], aps["outT"])

    const = ctx.enter_context(tc.tile_pool(name="const", bufs=1))
    xc_pool = ctx.enter_context(tc.tile_pool(name="xc", bufs=1))
    acc_pool = ctx.enter_context(tc.tile_pool(name="acc", bufs=1))
    wpool = ctx.enter_context(tc.tile_pool(name="w", bufs=3))
    htpool = ctx.enter_context(tc.tile_pool(name="ht", bufs=2))
    stage = ctx.enter_context(tc.tile_pool(name="stage", bufs=4))
    psA = ctx.enter_context(tc.tile_pool(name="psA", bufs=4, space="PSUM"))
    psB = ctx.enter_context(tc.tile_pool(name="psB", bufs=3, space="PSUM"))

    engs = [nc.sync, nc.gpsimd, nc.scalar]
    engs4 = [nc.sync, nc.gpsimd, nc.scalar, nc.vector]

    offs = []
    o = 0
    for ck in chunks:
        offs.append((o, ck))
        o += ck

    def issue_block_weights(hb):
        h0 = hb * HB
        w1_t = [wpool.tile([P, HB], MDT, tag=f"w1_{d}", name=f"w1t{d}")
                for d in range(DT)]
        w3_t = [wpool.tile([P, HB], MDT, tag=f"w3_{d}", name=f"w3t{d}")
                for d in range(DT)]
        for d in range(DT):
            nc.sync.dma_start(w1_t[d][:],
                              _ld(w1[d * P:(d + 1) * P, h0:h0 + HB], MDT))
            nc.gpsimd.dma_start(w3_t[d][:],
                                _ld(w3[d * P:(d + 1) * P, h0:h0 + HB], MDT))
        w2_t = [wpool.tile([P, D], MDT, tag=f"w2_{k}", name=f"w2t{k}")
                for k in range(HT)]
        for k in range(HT):
            nc.scalar.dma_start(w2_t[k][:],
                                _ld(w2[h0 + k * P:h0 + (k + 1) * P, :], MDT))
        return w1_t, w3_t, w2_t

    # ---- persistent activations ----
    # Critical-path-first DMA order: the first phase-A unit needs only
    # chunk 0 of every xc d-tile plus block-0 weights, so those land
    # first; the xc bulk and the gate inputs queue behind them.
    xc_t = [xc_pool.tile([P, C], MDT, tag=f"xc{d}", name=f"xc{d}")
            for d in range(DT)]
    c00, ck0 = offs[0]
    for d in range(DT):
        engs4[d % 4].dma_start(
            xc_t[d][:, c00:c00 + ck0],
            _ld(xc[d * P:(d + 1) * P, c00:c00 + ck0], MDT))
    pending = {0: issue_block_weights(0)}
    for (c0, ck) in offs[1:]:
        for d in range(DT):
            engs[d % 3].dma_start(
                xc_t[d][:, c0:c0 + ck],
                _ld(xc[d * P:(d + 1) * P, c0:c0 + ck], MDT))
    pending[1] = issue_block_weights(1)
    acc_t = [acc_pool.tile([P, C], F32, tag=f"acc{d}", name=f"acc{d}")
             for d in range(DT)]

    # gate inputs prefetched behind the xc bulk; consumed at GATE_HB
    wg_t = [const.tile([P, E], MDT, tag=f"wg{d}", name=f"wg{d}")
            for d in range(DT)]
    xg_t = [const.tile([P, TG], MDT, tag=f"xg{d}", name=f"xg{d}")
            for d in range(DT)]
    for d in range(DT):
        nc.scalar.dma_start(wg_t[d][:], _ld(wg[d * P:(d + 1) * P, :], MDT))
        nc.scalar.dma_start(xg_t[d][:],
                            _ld(xg[d * P:(d + 1) * P, :], MDT))

    for hb in range(NHB):
        w1_t, w3_t, w2_t = pending.pop(hb)
        if hb + 2 < NHB:
            pending[hb + 2] = issue_block_weights(hb + 2)

        # phase A: Ht[h, t] = silu(w1.T @ x) * (w3.T @ x) for this block
        ht_t = [htpool.tile([P, C], MDT, tag=f"ht{k}", name=f"htt{k}")
                for k in range(HT)]
        for (c0, ck) in offs:
            for k in range(HT):
                hsl = slice(k * P, (k + 1) * P)
                p1 = psA.tile([P, ck], F32, tag="p1", name="p1", bufs=3)
                p3 = psA.tile([P, ck], F32, tag="p3", name="p3", bufs=2)
                for d in range(DT):
                    nc.tensor.matmul(
                        p1[:], w1_t[d][:, hsl], xc_t[d][:, c0:c0 + ck],
                        start=(d == 0), stop=(d == DT - 1))
                for d in range(DT):
                    nc.tensor.matmul(
                        p3[:], w3_t[d][:, hsl], xc_t[d][:, c0:c0 + ck],
                        start=(d == 0), stop=(d == DT - 1))
                sil = stage.tile([P, ck], F32, tag="sil", name="sil")
                if _SIM_SAFE:
                    nc.scalar.activation(sil[:], p1[:], SIGMOID)
                    nc.vector.tensor_mul(sil[:], sil[:], p1[:])
                else:
                    nc.scalar.activation(sil[:], p1[:], SILU)
                nc.vector.tensor_mul(ht_t[k][:, c0:c0 + ck], sil[:], p3[:])

        if hb == GATE_HB:
            # gate compute tucked mid-pipeline (inputs prefetched at start)
            ps_g = psB.tile([E, TG], F32, tag="pb", name="psg")
            for d in range(DT):
                nc.tensor.matmul(ps_g[:], wg_t[d][:], xg_t[d][:],
                                 start=(d == 0), stop=(d == DT - 1))
            lg_s = const.tile([E, TG], F32, tag="lg", name="lg")
            nc.scalar.copy(lg_s[:], ps_g[:])
            nc.sync.dma_start(logits_o[:, :], lg_s[:])

        # phase B: outT[d, t] += w2.T @ Ht for this block
        last = hb == NHB - 1
        for dt in range(DT):
            dsl = slice(dt * P, (dt + 1) * P)
            for (c0, ck) in offs:
                pb = psB.tile([P, ck], F32, tag="pb", name="pb", bufs=3)
                for k in range(HT):
                    nc.tensor.matmul(
                        pb[:], w2_t[k][:, dsl], ht_t[k][:, c0:c0 + ck],
                        start=(k == 0), stop=(k == HT - 1))
                if hb == 0:
                    nc.vector.tensor_copy(acc_t[dt][:, c0:c0 + ck], pb[:])
                else:
                    nc.vector.tensor_add(acc_t[dt][:, c0:c0 + ck],
                                         acc_t[dt][:, c0:c0 + ck], pb[:])
            if last:
                engs[dt % 3].dma_start(outT_o[dt * P:(dt + 1) * P, :],
                                       acc_t[dt][:])


_NC_CACHE = {}
_LAST_EXEC_NS = None
_LAST_BR = None


def _build_nc(C):
    key = (C, _DTYPE)
    if key in _NC_CACHE:
        return _NC_CACHE[key]
    chunks = _chunks_of(C)
    mdt = F32 if _DTYPE == "f32r" else BF16
    nc = bacc.Bacc("TRN2", target_bir_lowering=False, debug=False,
                   num_devices=E)
    aps = {}
    for name, shape, dt in [("xg", [D, TG], mdt), ("wg", [D, E], mdt),
                            ("xc", [D, C], mdt), ("w1", [D, H], mdt),
                            ("w3", [D, H], mdt), ("w2", [H, D], mdt)]:
        aps[name] = nc.dram_tensor(name, shape, dt, kind="ExternalInput").ap()
    for name, shape in [("logits", [E, TG]), ("outT", [D, C])]:
        aps[name] = nc.dram_tensor(name, shape, F32, kind="ExternalOutput").ap()
    with tile.TileContext(nc) as tc:
        with ExitStack() as ctx:
            _moe_body(ctx, tc, aps, C, chunks)
    nc.compile()
    _NC_CACHE[key] = nc
    return nc


def kernel(x, wg, w1, w3, w2):
    x = np.asarray(x, np.float32)
    wg = np.asarray(wg, np.float32)
    w1 = np.asarray(w1, np.float32)
    w3 = np.asarray(w3, np.float32)
    w2 = np.asarray(w2, np.float32)
    xt = x.reshape(T, D)
    ndt = _np_mlp_dt()

    # host routing (indices only; combine weights come from device logits)
    lg_h = xt.astype(np.float64) @ wg.astype(np.float64)
    top2 = np.argsort(-lg_h, axis=1)[:, :2]                      # [T, 2]
    idx = [np.nonzero((top2 == e).any(axis=1))[0] for e in range(E)]
    counts = [len(i) for i in idx]
    C = max(512, ((max(counts) + 3) // 4) * 4)

    xT = np.ascontiguousarray(xt.T)                              # [D, T]
    nc = _build_nc(C)
    in_maps = []
    for e in range(E):
        xce = np.zeros((D, C), ndt)
        xce[:, :counts[e]] = xT[:, idx[e]].astype(ndt)
        in_maps.append({
            "xg": np.ascontiguousarray(xT[:, e * TG:(e + 1) * TG]).astype(ndt),
            "wg": wg.astype(ndt), "xc": xce,
            "w1": w1[e].astype(ndt, copy=False),
            "w3": w3[e].astype(ndt, copy=False),
            "w2": w2[e].astype(ndt, copy=False),
        })
    br = run_bass_kernel_spmd(nc, in_maps, list(range(E)))
    global _LAST_EXEC_NS, _LAST_BR
    _LAST_EXEC_NS = br.exec_time_ns
    _LAST_BR = br
    res = br.results

    # combine on host using device-computed gate logits
    lg = np.concatenate([res[e]["logits"].T for e in range(E)], axis=0)
    lg = lg - lg.max(axis=1, keepdims=True)
    p = np.exp(lg)
    p /= p.sum(axis=1, keepdims=True)
    pv = np.take_along_axis(p, top2, axis=1)                     # [T, 2]
    cw = (pv / pv.sum(axis=1, keepdims=True)).astype(np.float32)

    out = np.zeros((T, D), np.float32)
    for e in range(E):
        i = idx[e]
        we = np.where(top2[i, 0] == e, cw[i, 0], cw[i, 1])
        out[i] += we[:, None] * res[e]["outT"][:, :counts[e]].T
    return out.reshape(x.shape)


# revision 7
# speedup vs baseline: 1.1128x; 1.0248x over previous
"""MoE (top-2 of 8 experts, SwiGLU) Trainium2 kernel.

Strategy (expert-parallel, host-orchestrated dispatch):
  - Host computes routing (top-2 expert ids per token) from the gate logits
    and gathers each expert's tokens into a fixed-capacity buffer.
  - 8 NeuronCores run SPMD: core e holds expert e's weights, computes
      h = silu(x @ w1) * (x @ w3);  outT = (h @ w2)^T
    for its gathered tokens, plus a 1/8 slice of the gate logits
    (gate is data-parallel across cores).
  - Host combines: softmax over device-computed logits -> top-2 renormalized
    weights -> weighted scatter-add of per-expert outputs.

Layouts: activations are stored transposed (feature dim on partitions,
tokens on the free dim) so both matmul stages keep weights stationary:
  phase A: Ht[h, t]  = sum_d w1[d, h] * xT[d, t]   (lhsT = w1 tile)
  phase B: outT[d,t] = sum_h w2[h, d] * Ht[h, t]   (lhsT = w2 tile)
The MLP dtype is MOE_DTYPE (bf16 default | f32r); capacity C is the max
expert token count rounded up to 4 (not 128) to minimize padded columns.

Schedule notes (from NTFF trace of the f32r/C=1152 baseline: 28.7us
startup gap waiting on bulk xc+gate DMA ahead of block-0 weights, and a
14.5us tail storing all outputs after the last matmul):
  - DMA order: xc chunk 0 first (4 queues), then block-0/1 weights, then
    the xc remainder, then gate inputs; weight streams run 2 blocks ahead.
  - Final outT stores are interleaved into the last H-block's phase B so
    only the last d-tile's store trails compute.
"""

import os
from contextlib import ExitStack

import ml_dtypes
import numpy as np

import concourse.tile as tile
from concourse import bacc, mybir
import concourse.bass_utils as _bu
from concourse.bass_utils import run_bass_kernel_spmd

# If a caller enables BASS_TRACE, the trace path uploads NTFF artifacts to a
# shared bucket; containers without bucket access would crash the whole run.
# Fall back to the local tmpdir so tracing still completes.
_orig_upload = _bu.upload_artifacts


def _safe_upload(tmpdir):
    try:
        return _orig_upload(tmpdir)
    except Exception:
        return tmpdir


_bu.upload_artifacts = _safe_upload

P = 128
D = 1024
H = 4096
E = 8
T = 4096
TG = T // E  # gate tokens per core (data-parallel gate)
HB = 256     # H block size (weights streamed block-by-block)
GATE_HB = 3  # H block whose slot hides the gate matmul
F32 = mybir.dt.float32
F32R = mybir.dt.float32r
BF16 = mybir.dt.bfloat16
SIGMOID = mybir.ActivationFunctionType.Sigmoid
SILU = mybir.ActivationFunctionType.Silu
# CoreSim does not implement Silu; set MOE_SIM_SAFE=1 to emit sigmoid*x.
_SIM_SAFE = os.environ.get("MOE_SIM_SAFE") == "1"
# MLP matmul dtype: "bf16" (default, ~4.5e-3 rel err) or "f32r" (~2.8e-4)
_DTYPE = os.environ.get("MOE_DTYPE", "bf16")


def _mlp_dt():
    return F32R if _DTYPE == "f32r" else BF16


def _np_mlp_dt():
    return np.float32 if _DTYPE == "f32r" else ml_dtypes.bfloat16


def _chunks_of(c):
    """Split capacity C into near-equal matmul free-dim chunks.

    Each chunk is a multiple of 4 and at most 512 (one f32 PSUM bank);
    near-equal sizes keep LDWEIGHTS hidden under column streaming.
    """
    if c < 512 or c % 4 != 0:
        raise ValueError(f"bad capacity {c}")
    n = -(-c // 512)
    q = c // 4
    base, extra = divmod(q, n)
    return [4 * (base + (1 if i < extra else 0)) for i in range(n)]


def _ld(ap, dt):
    """DRAM-side AP for a weight/activation load at the MLP dtype."""
    return ap.bitcast(dt) if dt == F32R else ap


def _moe_body(ctx, tc, aps, C, chunks):
    nc = tc.nc
    MDT = _mlp_dt()
    DT = D // P        # 8 d-tiles
    HT = HB // P       # h-tiles per block
    NHB = H // HB      # number of H blocks
    xg, wg, xc, w1, w3, w2, logits_o, outT_o = (
        aps["xg"], aps["wg"], aps["xc"], aps["w1"], aps["w3"], aps["w2"],
        aps["logits"], aps["outT"])

    const = ctx.enter_context(tc.tile_pool(name="const", bufs=1))
    xc_pool = ctx.enter_context(tc.tile_pool(name="xc", bufs=1))
    acc_pool = ctx.enter_context(tc.tile_pool(name="acc", bufs=1))
    wpool = ctx.enter_context(tc.tile_pool(name="w", bufs=3))
    htpool = ctx.enter_context(tc.tile_pool(name="ht", bufs=2))
    stage = ctx.enter_context(tc.tile_pool(name="stage", bufs=4))
    psA = ctx.enter_context(tc.tile_pool(name="psA", bufs=4, space="PSUM"))
    psB = ctx.enter_context(tc.tile_pool(name="psB", bufs=3, space="PSUM"))

    engs = [nc.sync, nc.gpsimd, nc.scalar]

    offs = []
    o = 0
    for ck in chunks:
        offs.append((o, ck))
        o += ck

    def issue_block_weights(hb, interleave=False):
        h0 = hb * HB
        w1_t = [wpool.tile([P, HB], MDT, tag=f"w1_{d}", name=f"w1t{d}")
                for d in range(DT)]
        w3_t = [wpool.tile([P, HB], MDT, tag=f"w3_{d}", name=f"w3t{d}")
                for d in range(DT)]
        if interleave:
            # block 0 startup: w1 lands d-ascending across two queues so
            # the first accumulation group can start streaming early
            for d in range(DT):
                [nc.sync, nc.gpsimd][d % 2].dma_start(
                    w1_t[d][:], _ld(w1[d * P:(d + 1) * P, h0:h0 + HB], MDT))
            for d in range(DT):
                [nc.gpsimd, nc.sync][d % 2].dma_start(
                    w3_t[d][:], _ld(w3[d * P:(d + 1) * P, h0:h0 + HB], MDT))
        else:
            for d in range(DT):
                nc.sync.dma_start(w1_t[d][:],
                                  _ld(w1[d * P:(d + 1) * P, h0:h0 + HB], MDT))
                nc.gpsimd.dma_start(w3_t[d][:],
                                    _ld(w3[d * P:(d + 1) * P, h0:h0 + HB], MDT))
        w2_t = [wpool.tile([P, D], MDT, tag=f"w2_{k}", name=f"w2t{k}")
                for k in range(HT)]
        for k in range(HT):
            nc.scalar.dma_start(w2_t[k][:],
                                _ld(w2[h0 + k * P:h0 + (k + 1) * P, :], MDT))
        return w1_t, w3_t, w2_t

    # ---- persistent activations ----
    # Critical-path-first DMA order: the first phase-A unit needs only
    # chunk 0 of every xc d-tile plus block-0 weights, so those land
    # first; the xc bulk and the gate inputs queue behind them.
    xc_t = [xc_pool.tile([P, C], MDT, tag=f"xc{d}", name=f"xc{d}")
            for d in range(DT)]
    c00, ck0 = offs[0]
    for d in range(DT):
        engs[d % 3].dma_start(
            xc_t[d][:, c00:c00 + ck0],
            _ld(xc[d * P:(d + 1) * P, c00:c00 + ck0], MDT))
    pending = {0: issue_block_weights(0, interleave=True)}
    for (c0, ck) in offs[1:]:
        for d in range(DT):
            engs[d % 3].dma_start(
                xc_t[d][:, c0:c0 + ck],
                _ld(xc[d * P:(d + 1) * P, c0:c0 + ck], MDT))
    pending[1] = issue_block_weights(1)
    acc_t = [acc_pool.tile([P, C], F32, tag=f"acc{d}", name=f"acc{d}")
             for d in range(DT)]

    # gate inputs prefetched behind the xc bulk; consumed at GATE_HB
    wg_t = [const.tile([P, E], MDT, tag=f"wg{d}", name=f"wg{d}")
            for d in range(DT)]
    xg_t = [const.tile([P, TG], MDT, tag=f"xg{d}", name=f"xg{d}")
            for d in range(DT)]
    for d in range(DT):
        nc.scalar.dma_start(wg_t[d][:], _ld(wg[d * P:(d + 1) * P, :], MDT))
        nc.scalar.dma_start(xg_t[d][:],
                            _ld(xg[d * P:(d + 1) * P, :], MDT))

    for hb in range(NHB):
        w1_t, w3_t, w2_t = pending.pop(hb)
        if hb + 2 < NHB:
            pending[hb + 2] = issue_block_weights(hb + 2)

        # phase A: Ht[h, t] = silu(w1.T @ x) * (w3.T @ x) for this block
        ht_t = [htpool.tile([P, C], MDT, tag=f"ht{k}", name=f"htt{k}")
                for k in range(HT)]
        for (c0, ck) in offs:
            for k in range(HT):
                hsl = slice(k * P, (k + 1) * P)
                p1 = psA.tile([P, ck], F32, tag="p1", name="p1", bufs=3)
                p3 = psA.tile([P, ck], F32, tag="p3", name="p3", bufs=2)
                for d in range(DT):
                    nc.tensor.matmul(
                        p1[:], w1_t[d][:, hsl], xc_t[d][:, c0:c0 + ck],
                        start=(d == 0), stop=(d == DT - 1))
                for d in range(DT):
                    nc.tensor.matmul(
                        p3[:], w3_t[d][:, hsl], xc_t[d][:, c0:c0 + ck],
                        start=(d == 0), stop=(d == DT - 1))
                sil = stage.tile([P, ck], F32, tag="sil", name="sil")
                if _SIM_SAFE:
                    nc.scalar.activation(sil[:], p1[:], SIGMOID)
                    nc.vector.tensor_mul(sil[:], sil[:], p1[:])
                else:
                    nc.scalar.activation(sil[:], p1[:], SILU)
                nc.vector.tensor_mul(ht_t[k][:, c0:c0 + ck], sil[:], p3[:])

        if hb == GATE_HB:
            # gate compute tucked mid-pipeline (inputs prefetched at start)
            ps_g = psB.tile([E, TG], F32, tag="pb", name="psg")
            for d in range(DT):
                nc.tensor.matmul(ps_g[:], wg_t[d][:], xg_t[d][:],
                                 start=(d == 0), stop=(d == DT - 1))
            lg_s = const.tile([E, TG], F32, tag="lg", name="lg")
            nc.scalar.copy(lg_s[:], ps_g[:])
            nc.sync.dma_start(logits_o[:, :], lg_s[:])

        # phase B: outT[d, t] += w2.T @ Ht for this block
        last = hb == NHB - 1
        for dt in range(DT):
            dsl = slice(dt * P, (dt + 1) * P)
            for ci, (c0, ck) in enumerate(offs):
                pb = psB.tile([P, ck], F32, tag="pb", name="pb", bufs=3)
                for k in range(HT):
                    nc.tensor.matmul(
                        pb[:], w2_t[k][:, dsl], ht_t[k][:, c0:c0 + ck],
                        start=(k == 0), stop=(k == HT - 1))
                if hb == 0:
                    nc.vector.tensor_copy(acc_t[dt][:, c0:c0 + ck], pb[:])
                else:
                    nc.vector.tensor_add(acc_t[dt][:, c0:c0 + ck],
                                         acc_t[dt][:, c0:c0 + ck], pb[:])
                if last:
                    # stream finished output slices out as they complete
                    engs[(dt + ci) % 3].dma_start(
                        outT_o[dt * P:(dt + 1) * P, c0:c0 + ck],
                        acc_t[dt][:, c0:c0 + ck])


_NC_CACHE = {}
_LAST_EXEC_NS = None
_LAST_BR = None


def _build_nc(C):
    key = (C, _DTYPE)
    if key in _NC_CACHE:
        return _NC_CACHE[key]
    chunks = _chunks_of(C)
    mdt = F32 if _DTYPE == "f32r" else BF16
    nc = bacc.Bacc("TRN2", target_bir_lowering=False, debug=False,
                   num_devices=E)
    aps = {}
    for name, shape, dt in [("xg", [D, TG], mdt), ("wg", [D, E], mdt),
                            ("xc", [D, C], mdt), ("w1", [D, H], mdt),
                            ("w3", [D, H], mdt), ("w2", [H, D], mdt)]:
        aps[name] = nc.dram_tensor(name, shape, dt, kind="ExternalInput").ap()
    for name, shape in [("logits", [E, TG]), ("outT", [D, C])]:
        aps[name] = nc.dram_tensor(name, shape, F32, kind="ExternalOutput").ap()
    with tile.TileContext(nc) as tc:
        with ExitStack() as ctx:
            _moe_body(ctx, tc, aps, C, chunks)
    nc.compile()
    _NC_CACHE[key] = nc
    return nc


def kernel(x, wg, w1, w3, w2):
    x = np.asarray(x, np.float32)
    wg = np.asarray(wg, np.float32)
    w1 = np.asarray(w1, np.float32)
    w3 = np.asarray(w3, np.float32)
    w2 = np.asarray(w2, np.float32)
    xt = x.reshape(T, D)
    ndt = _np_mlp_dt()

    # host routing (indices only; combine weights come from device logits)
    lg_h = xt.astype(np.float64) @ wg.astype(np.float64)
    top2 = np.argsort(-lg_h, axis=1)[:, :2]                      # [T, 2]
    idx = [np.nonzero((top2 == e).any(axis=1))[0] for e in range(E)]
    counts = [len(i) for i in idx]
    C = max(512, ((max(counts) + 3) // 4) * 4)

    xT = np.ascontiguousarray(xt.T)                              # [D, T]
    nc = _build_nc(C)
    in_maps = []
    for e in range(E):
        xce = np.zeros((D, C), ndt)
        xce[:, :counts[e]] = xT[:, idx[e]].astype(ndt)
        in_maps.append({
            "xg": np.ascontiguousarray(xT[:, e * TG:(e + 1) * TG]).astype(ndt),
            "wg": wg.astype(ndt), "xc": xce,
            "w1": w1[e].astype(ndt, copy=False),
            "w3": w3[e].astype(ndt, copy=False),
            "w2": w2[e].astype(ndt, copy=False),
        })
    br = run_bass_kernel_spmd(nc, in_maps, list(range(E)))
    global _LAST_EXEC_NS, _LAST_BR
    _LAST_EXEC_NS = br.exec_time_ns
    _LAST_BR = br
    res = br.results

    # combine on host using device-computed gate logits
    lg = np.concatenate([res[e]["logits"].T for e in range(E)], axis=0)
    lg = lg - lg.max(axis=1, keepdims=True)
    p = np.exp(lg)
    p /= p.sum(axis=1, keepdims=True)
    pv = np.take_along_axis(p, top2, axis=1)                     # [T, 2]
    cw = (pv / pv.sum(axis=1, keepdims=True)).astype(np.float32)

    out = np.zeros((T, D), np.float32)
    for e in range(E):
        i = idx[e]
        we = np.where(top2[i, 0] == e, cw[i, 0], cw[i, 1])
        out[i] += we[:, None] * res[e]["outT"][:, :counts[e]].T
    return out.reshape(x.shape)


# revision 9
# speedup vs baseline: 1.1906x; 1.0699x over previous
"""MoE (top-2 of 8 experts, SwiGLU) Trainium2 kernel.

Strategy (expert-parallel, host-orchestrated dispatch):
  - Host computes routing (top-2 expert ids per token) from the gate logits
    and gathers each expert's tokens into a fixed-capacity buffer.
  - 8 NeuronCores run SPMD: core e holds expert e's weights, computes
      h = silu(x @ w1) * (x @ w3);  outT = (h @ w2)^T
    for its gathered tokens, plus a 1/8 slice of the gate logits
    (gate is data-parallel across cores).
  - Host combines: softmax over device-computed logits -> top-2 renormalized
    weights -> weighted scatter-add of per-expert outputs.

Layouts: activations are stored transposed (feature dim on partitions,
tokens on the free dim) so both matmul stages keep weights stationary:
  phase A: Ht[h, t]  = sum_d w1[d, h] * xT[d, t]   (lhsT = w1 tile)
  phase B: outT[d,t] = sum_h w2[h, d] * Ht[h, t]   (lhsT = w2 tile)
The MLP dtype is MOE_DTYPE (bf16 default | f32r); capacity C is the max
expert token count rounded up to 4 to minimize padded columns.

Schedule notes (from NTFF traces of earlier revisions):
  - All DRAM inputs are host-packed into the exact SBUF layout so every
    weight block / activation chunk is ONE contiguous DMA. DMA issue
    occupies the issuing sequencer ~0.6us, and the Scalar sequencer also
    runs the phase-A silu, so scalar stays DMA-free during compute.
  - Startup: xc chunk 0 and the block-0 w1/w3 are split across the
    sync/gpsimd/scalar queues so the first matmul starts ~4us in.
  - Phase B accumulates over PAIRS of H-blocks (4 matmuls per PSUM
    group) halving accumulate traffic on the vector engine.
  - The final accumulate writes bf16 output tiles directly; stores are
    issued per d-tile as they complete so only the last ~0.3MB trails.
"""

import os
from contextlib import ExitStack

import ml_dtypes
import numpy as np

import concourse.tile as tile
from concourse import bacc, mybir
import concourse.bass_utils as _bu
from concourse.bass_utils import run_bass_kernel_spmd

# If a caller enables BASS_TRACE, the trace path uploads NTFF artifacts to a
# shared bucket; containers without bucket access would crash the whole run.
# Fall back to the local tmpdir so tracing still completes.
_orig_upload = _bu.upload_artifacts


def _safe_upload(tmpdir):
    try:
        return _orig_upload(tmpdir)
    except Exception:
        return tmpdir


_bu.upload_artifacts = _safe_upload

P = 128
D = 1024
H = 4096
E = 8
T = 4096
TG = T // E  # gate tokens per core (data-parallel gate)
HB = 256     # H block size (weights streamed block-by-block)
DT = D // P  # 8 d-tiles
HT = HB // P  # h-tiles per block
NHB = H // HB  # number of H blocks
GATE_HB = 3  # H block whose slot hides the gate matmul
F32 = mybir.dt.float32
F32R = mybir.dt.float32r
BF16 = mybir.dt.bfloat16
SIGMOID = mybir.ActivationFunctionType.Sigmoid
SILU = mybir.ActivationFunctionType.Silu
# CoreSim does not implement Silu; set MOE_SIM_SAFE=1 to emit sigmoid*x.
_SIM_SAFE = os.environ.get("MOE_SIM_SAFE") == "1"
# MLP matmul dtype: "bf16" (default, ~4.5e-3 rel err) or "f32r" (~2.8e-4)
_DTYPE = os.environ.get("MOE_DTYPE", "bf16")


def _mlp_dt():
    return F32R if _DTYPE == "f32r" else BF16


def _np_mlp_dt():
    return np.float32 if _DTYPE == "f32r" else ml_dtypes.bfloat16


def _chunks_of(c):
    """Split capacity C into near-equal matmul free-dim chunks.

    Each chunk is a multiple of 4 and at most 512 (one f32 PSUM bank);
    near-equal sizes keep LDWEIGHTS hidden under column streaming.
    """
    if c < 512 or c % 4 != 0:
        raise ValueError(f"bad capacity {c}")
    n = -(-c // 512)
    q = c // 4
    base, extra = divmod(q, n)
    return [4 * (base + (1 if i < extra else 0)) for i in range(n)]


def _ld(ap, dt):
    """DRAM-side AP for a weight/activation load at the MLP dtype."""
    return ap.bitcast(dt) if dt == F32R else ap


def _moe_body(ctx, tc, aps, C, chunks):
    nc = tc.nc
    MDT = _mlp_dt()
    ODT = F32 if _DTYPE == "f32r" else BF16
    xg, wg, xc, w1, w3, w2, logits_o, outT_o = (
        aps["xg"], aps["wg"], aps["xc"], aps["w1"], aps["w3"], aps["w2"],
        aps["logits"], aps["outT"])

    const = ctx.enter_context(tc.tile_pool(name="const", bufs=1))
    xc_pool = ctx.enter_context(tc.tile_pool(name="xc", bufs=1))
    acc_pool = ctx.enter_context(tc.tile_pool(name="acc", bufs=1))
    opool = ctx.enter_context(tc.tile_pool(name="out", bufs=1))
    wpool = ctx.enter_context(tc.tile_pool(name="w", bufs=4))
    htpool = ctx.enter_context(tc.tile_pool(name="ht", bufs=2))
    stage = ctx.enter_context(tc.tile_pool(name="stage", bufs=4))
    psA = ctx.enter_context(tc.tile_pool(name="psA", bufs=4, space="PSUM"))
    psB = ctx.enter_context(tc.tile_pool(name="psB", bufs=3, space="PSUM"))

    S, G, SC = nc.sync, nc.gpsimd, nc.scalar
    engs = [S, G, SC]

    offs = []
    o = 0
    for ck in chunks:
        offs.append((o, ck))
        o += ck

    # ---- tiles ----
    # packed layouts (host pre-arranged, one contiguous DMA per load):
    #   xc  [P, 8*C]      chunk-major: [:, 8*c0 + d*ck + j]
    #   w1  [NHB*P, 8*HB] block rows:  [hb*P + p, d*HB + h]
    #   w2  [NHB*P, HT*D] block rows:  [hb*P + p, k*D + dcol]
    #   xg  [P, DT*TG], wg [P, DT*E]
    xc_t = xc_pool.tile([P, DT * C], MDT, tag="xc", name="xc")
    acc_t = [acc_pool.tile([P, C], F32, tag=f"acc{d}", name=f"acc{d}")
             for d in range(DT)]
    out_t = [opool.tile([P, C], ODT, tag=f"out{d}", name=f"out{d}")
             for d in range(DT)]
    wg_t = const.tile([P, DT * E], MDT, tag="wg", name="wg")
    xg_t = const.tile([P, DT * TG], MDT, tag="xg", name="xg")

    def wtiles(hb):
        w1_t = wpool.tile([P, DT * HB], MDT, tag="w1", name=f"w1b{hb}")
        w3_t = wpool.tile([P, DT * HB], MDT, tag="w3", name=f"w3b{hb}")
        w2_t = wpool.tile([P, HT * D], MDT, tag="w2", name=f"w2b{hb}")
        return w1_t, w3_t, w2_t

    def wrows(hb):
        return slice(hb * P, (hb + 1) * P)

    # ---- prologue DMA (priority order; balanced across 3 queues) ----
    pending = {0: wtiles(0), 1: wtiles(1)}

    # xc chunks in thirds, block-0 w1/w3 halves interleaved after chunk 0
    for ci in range(len(offs)):
        c0, ck = offs[ci]
        b = DT * c0
        S.dma_start(xc_t[:, b:b + 3 * ck], _ld(xc[:, b:b + 3 * ck], MDT))
        G.dma_start(xc_t[:, b + 3 * ck:b + 6 * ck],
                    _ld(xc[:, b + 3 * ck:b + 6 * ck], MDT))
        SC.dma_start(xc_t[:, b + 6 * ck:b + 8 * ck],
                     _ld(xc[:, b + 6 * ck:b + 8 * ck], MDT))
        if ci == 0:
            w1_0, w3_0, w2_0 = pending[0]
            half = DT * HB // 2
            S.dma_start(w1_0[:, :half], _ld(w1[wrows(0), :half], MDT))
            G.dma_start(w1_0[:, half:], _ld(w1[wrows(0), half:], MDT))
            S.dma_start(w3_0[:, :half], _ld(w3[wrows(0), :half], MDT))
            G.dma_start(w3_0[:, half:], _ld(w3[wrows(0), half:], MDT))
    w1_1, w3_1, w2_1 = pending[1]
    SC.dma_start(w2_0[:], _ld(w2[wrows(0), :], MDT))
    SC.dma_start(w2_1[:], _ld(w2[wrows(1), :], MDT))
    S.dma_start(w1_1[:], _ld(w1[wrows(1), :], MDT))
    G.dma_start(w3_1[:], _ld(w3[wrows(1), :], MDT))
    S.dma_start(wg_t[:], _ld(wg[:, :], MDT))
    S.dma_start(xg_t[:], _ld(xg[:, :], MDT))

    def issue_block(hb, w2q):
        w1_t, w3_t, w2_t = wtiles(hb)
        S.dma_start(w1_t[:], _ld(w1[wrows(hb), :], MDT))
        G.dma_start(w3_t[:], _ld(w3[wrows(hb), :], MDT))
        w2q.dma_start(w2_t[:], _ld(w2[wrows(hb), :], MDT))
        return w1_t, w3_t, w2_t

    # ---- main loop over H-block pairs ----
    for pr in range(NHB // 2):
        blocks = (2 * pr, 2 * pr + 1)
        cur = [pending.pop(hb) for hb in blocks]
        for j, hb in enumerate((2 * pr + 2, 2 * pr + 3)):
            if hb < NHB:
                pending[hb] = issue_block(hb, [S, G][j])

        # phase A for both blocks of the pair
        ht_t = [htpool.tile([P, C], MDT, tag=f"ht{i}", name=f"htt{i}")
                for i in range(2 * HT)]
        for b, hb in enumerate(blocks):
            w1_t, w3_t, _ = cur[b]
            for (c0, ck) in offs:
                xb = DT * c0
                for k in range(HT):
                    p1 = psA.tile([P, ck], F32, tag="p1", name="p1", bufs=3)
                    p3 = psA.tile([P, ck], F32, tag="p3", name="p3", bufs=2)
                    for d in range(DT):
                        nc.tensor.matmul(
                            p1[:], w1_t[:, d * HB + k * P:d * HB + (k + 1) * P],
                            xc_t[:, xb + d * ck:xb + (d + 1) * ck],
                            start=(d == 0), stop=(d == DT - 1))
                    for d in range(DT):
                        nc.tensor.matmul(
                            p3[:], w3_t[:, d * HB + k * P:d * HB + (k + 1) * P],
                            xc_t[:, xb + d * ck:xb + (d + 1) * ck],
                            start=(d == 0), stop=(d == DT - 1))
                    sil = stage.tile([P, ck], F32, tag="sil", name="sil")
                    if _SIM_SAFE:
                        nc.scalar.activation(sil[:], p1[:], SIGMOID)
                        nc.vector.tensor_mul(sil[:], sil[:], p1[:])
                    else:
                        nc.scalar.activation(sil[:], p1[:], SILU)
                    nc.vector.tensor_mul(
                        ht_t[b * HT + k][:, c0:c0 + ck], sil[:], p3[:])
            if hb == GATE_HB:
                # gate compute tucked mid-pipeline (inputs prefetched early)
                ps_g = psB.tile([E, TG], F32, tag="pb", name="psg")
                for d in range(DT):
                    nc.tensor.matmul(
                        ps_g[:], wg_t[:, d * E:(d + 1) * E],
                        xg_t[:, d * TG:(d + 1) * TG],
                        start=(d == 0), stop=(d == DT - 1))
                lg_s = const.tile([E, TG], F32, tag="lg", name="lg")
                nc.scalar.copy(lg_s[:], ps_g[:])
                S.dma_start(logits_o[:, :], lg_s[:])

        # phase B: outT[d, t] += w2.T @ Ht over the pair's 4 h-tiles
        first, last = pr == 0, pr == NHB // 2 - 1
        for dt in range(DT):
            for (c0, ck) in offs:
                pb = psB.tile([P, ck], F32, tag="pb", name="pb", bufs=3)
                for i in range(2 * HT):
                    b, k = divmod(i, HT)
                    nc.tensor.matmul(
                        pb[:], cur[b][2][:, k * D + dt * P:k * D + (dt + 1) * P],
                        ht_t[i][:, c0:c0 + ck],
                        start=(i == 0), stop=(i == 2 * HT - 1))
                if first:
                    nc.vector.tensor_copy(acc_t[dt][:, c0:c0 + ck], pb[:])
                elif last:
                    nc.vector.tensor_add(out_t[dt][:, c0:c0 + ck],
                                         acc_t[dt][:, c0:c0 + ck], pb[:])
                else:
                    nc.vector.tensor_add(acc_t[dt][:, c0:c0 + ck],
                                         acc_t[dt][:, c0:c0 + ck], pb[:])
            if last:
                # stream finished output slices out as they complete
                engs[dt % 3].dma_start(outT_o[dt * P:(dt + 1) * P, :],
                                       out_t[dt][:])


_NC_CACHE = {}
_LAST_EXEC_NS = None
_LAST_BR = None


def _build_nc(C):
    key = (C, _DTYPE)
    if key in _NC_CACHE:
        return _NC_CACHE[key]
    chunks = _chunks_of(C)
    mdt = F32 if _DTYPE == "f32r" else BF16
    odt = F32 if _DTYPE == "f32r" else BF16
    nc = bacc.Bacc("TRN2", target_bir_lowering=False, debug=False,
                   num_devices=E)
    aps = {}
    for name, shape in [("xg", [P, DT * TG]), ("wg", [P, DT * E]),
                        ("xc", [P, DT * C]), ("w1", [NHB * P, DT * HB]),
                        ("w3", [NHB * P, DT * HB]), ("w2", [NHB * P, HT * D])]:
        aps[name] = nc.dram_tensor(name, shape, mdt,
                                   kind="ExternalInput").ap()
    aps["logits"] = nc.dram_tensor("logits", [E, TG], F32,
                                   kind="ExternalOutput").ap()
    aps["outT"] = nc.dram_tensor("outT", [D, C], odt,
                                 kind="ExternalOutput").ap()
    with tile.TileContext(nc) as tc:
        with ExitStack() as ctx:
            _moe_body(ctx, tc, aps, C, chunks)
    nc.compile()
    _NC_CACHE[key] = nc
    return nc


def _pack_dtiles(a2d, ndt):
    """[DT*P, W] -> [P, DT*W] with d-tiles side by side in the free dim."""
    dp, w = a2d.shape
    return np.ascontiguousarray(
        a2d.reshape(DT, P, w).transpose(1, 0, 2).reshape(P, DT * w)
    ).astype(ndt, copy=False)


def kernel(x, wg, w1, w3, w2):
    x = np.asarray(x, np.float32)
    wg = np.asarray(wg, np.float32)
    w1 = np.asarray(w1, np.float32)
    w3 = np.asarray(w3, np.float32)
    w2 = np.asarray(w2, np.float32)
    xt = x.reshape(T, D)
    ndt = _np_mlp_dt()

    # host routing (indices only; combine weights come from device logits)
    lg_h = xt.astype(np.float64) @ wg.astype(np.float64)
    top2 = np.argsort(-lg_h, axis=1)[:, :2]                      # [T, 2]
    idx = [np.nonzero((top2 == e).any(axis=1))[0] for e in range(E)]
    counts = [len(i) for i in idx]
    C = max(512, ((max(counts) + 3) // 4) * 4)
    chunks = _chunks_of(C)

    xT = np.ascontiguousarray(xt.T)                              # [D, T]
    nc = _build_nc(C)
    wg_p = _pack_dtiles(wg, ndt)                                 # [P, DT*E]
    in_maps = []
    for e in range(E):
        xce = np.zeros((D, C), np.float32)
        xce[:, :counts[e]] = xT[:, idx[e]]
        xce3 = xce.reshape(DT, P, C)
        xcp = np.empty((P, DT * C), ndt)
        o = 0
        for ck in chunks:
            xcp[:, DT * o:DT * (o + ck)] = (
                xce3[:, :, o:o + ck].transpose(1, 0, 2).reshape(P, DT * ck))
            o += ck
        # w1/w3: [D, H] -> [NHB*P, DT*HB]; w2: [H, D] -> [NHB*P, HT*D]
        w1p = np.ascontiguousarray(
            w1[e].reshape(DT, P, NHB, HB).transpose(2, 1, 0, 3)
            .reshape(NHB * P, DT * HB)).astype(ndt, copy=False)
        w3p = np.ascontiguousarray(
            w3[e].reshape(DT, P, NHB, HB).transpose(2, 1, 0, 3)
            .reshape(NHB * P, DT * HB)).astype(ndt, copy=False)
        w2p = np.ascontiguousarray(
            w2[e].reshape(NHB, HT, P, D).transpose(0, 2, 1, 3)
            .reshape(NHB * P, HT * D)).astype(ndt, copy=False)
        in_maps.append({
            "xg": _pack_dtiles(xT[:, e * TG:(e + 1) * TG], ndt),
            "wg": wg_p, "xc": xcp, "w1": w1p, "w3": w3p, "w2": w2p,
        })
    br = run_bass_kernel_spmd(nc, in_maps, list(range(E)))
    global _LAST_EXEC_NS, _LAST_BR
    _LAST_EXEC_NS = br.exec_time_ns
    _LAST_BR = br
    res = br.results

    # combine on host using device-computed gate logits
    lg = np.concatenate([res[e]["logits"].T for e in range(E)], axis=0)
    lg = lg - lg.max(axis=1, keepdims=True)
    p = np.exp(lg)
    p /= p.sum(axis=1, keepdims=True)
    pv = np.take_along_axis(p, top2, axis=1)                     # [T, 2]
    cw = (pv / pv.sum(axis=1, keepdims=True)).astype(np.float32)

    out = np.zeros((T, D), np.float32)
    for e in range(E):
        i = idx[e]
        we = np.where(top2[i, 0] == e, cw[i, 0], cw[i, 1])
        out[i] += we[:, None] * res[e]["outT"][:, :counts[e]].astype(np.float32).T
    return out.reshape(x.shape)
